# revision 5
# baseline (speedup 1.0000x reference)
"""BSplineKAN layer forward on 8 Trainium2 NeuronCores (Bass/Tile).

out = silu(x @ base_weight) + einsum('bir,ior->bo', bspline_basis(x), coeff)

Math: with uniform knots t_j = t3 + (j-3)*h (t3 = left clamp bound) and
s = (clamp(x) - t3)/h in [0, G], the cubic B-spline basis functions are exact
linear combinations of 8 bounded one-sided cubes:
    L_k = max(s-k, 0)^3,  R_k = min(s-k, 0)*(s-k)^2   (k = 1..4)
The (8 basis <- 8 features) linear map M is solved on the host in float64 and
folded into the coeff tensor:  W2[(f,i), o] = sum_r M[f,r] * coeff[i,o,r].
The device kernel is then 9 dense matmuls per batch tile (K = 8*512 spline +
512 base) plus cheap elementwise feature construction:
    d_k = s-k (DVE tensor_scalar), q_k = d_k^2 (ACT Square),
    L_k = max(d_k,0)*q_k, R_k = min(d_k,0)*q_k (DVE scalar_tensor_tensor).

Sharding: data-parallel over batch; each of 8 cores handles 2048 rows with
replicated weights. No collectives needed.

Matmuls run in fp16 (11-bit mantissa ~= TF32; measured end-to-end error
~3e-4 scale-relative, ~3x faster than fp32r and ~6x faster than fp32 on PE).
"""

import numpy as np

import concourse.bass as bass
import concourse.mybir as mybir
import concourse.tile as tile
from concourse import bacc
from concourse.bass_utils import run_bass_kernel_spmd
from concourse.masks import make_identity

N_CORES = 8
BATCH, N_IN, N_OUT = 16384, 512, 512
SPLINE_ORDER, N_GRID = 3, 5
N_BASIS = N_GRID + SPLINE_ORDER  # 8
B_CORE = BATCH // N_CORES        # 2048
WINDOW = 1024                    # batch columns per feature window
N_IC = N_IN // 128               # 4 contraction chunks per feature
N_FEAT = 8

f32 = mybir.dt.float32
f16 = mybir.dt.float16
bf16 = mybir.dt.bfloat16
AF = mybir.ActivationFunctionType
ALU = mybir.AluOpType

MM_DT = f16  # matmul dtype for weights/features (f16 or bf16)


# ----------------------------------------------------------------------------
# Host-side math
# ----------------------------------------------------------------------------

def _bspline_basis_f64(x, knots):
    """Cox-de Boor recursion (float64), matching the reference semantics."""
    t = knots.astype(np.float64)
    xc = np.clip(x.astype(np.float64), t[SPLINE_ORDER], t[-SPLINE_ORDER - 1])[..., None]
    n_int = len(t) - 1
    B = ((xc >= t[:-1]) & (xc < t[1:])).astype(np.float64)
    for j in range(1, SPLINE_ORDER + 1):
        nv = n_int - j
        ti = t[:nv]
        ti_j = t[j:nv + j]
        ti1 = t[1:nv + 1]
        ti_j1 = t[j + 1:nv + j + 1]
        a1 = (xc - ti) / np.maximum(ti_j - ti, 1e-8)
        a2 = (ti_j1 - xc) / np.maximum(ti_j1 - ti1, 1e-8)
        B = a1 * B[..., :nv] + a2 * B[..., 1:nv + 1]
    return B  # (..., N_BASIS)


def _features_f64(x, t3, h):
    """The 8 one-sided cubes of s = (clamp(x)-t3)/h. Order: L1 R1 L2 R2 L3 R3 L4 R4."""
    G = N_GRID
    s = (np.clip(x.astype(np.float64), t3, t3 + G * h) - t3) / h
    feats = []
    for k in range(1, 5):
        d = s - k
        q = d * d
        feats.append(np.maximum(d, 0.0) * q)   # L_k
        feats.append(np.minimum(d, 0.0) * q)   # R_k (negated one-sided cube)
    return np.stack(feats, axis=-1)  # (..., 8)


def _solve_basis_map(knots):
    """M (8 feats x 8 basis) with basis = features @ M, solved exactly in f64."""
    t3 = float(knots[SPLINE_ORDER])
    h = float(knots[SPLINE_ORDER + 1] - knots[SPLINE_ORDER])
    g = np.linspace(t3 - 0.5, t3 + N_GRID * h + 0.5, 4001)
    g = np.concatenate([g, knots.astype(np.float64), [t3, t3 + N_GRID * h]])
    F = _features_f64(g, t3, h)              # (n, 8)
    Bref = _bspline_basis_f64(g, knots)      # (n, 8)
    M, res, _, _ = np.linalg.lstsq(F, Bref, rcond=None)
    err = np.abs(F @ M - Bref).max()
    # knots come in as float32 and are not exactly uniform, so the closed-form
    # uniform features reproduce the reference basis only to ~1e-7.
    if err > 1e-5:
        raise ValueError(f"basis map residual too large: {err}")
    return M, t3, h


# ----------------------------------------------------------------------------
# Device kernel (one SPMD program for all 8 cores)
# ----------------------------------------------------------------------------

def _build_nc(inv_h, s_bias, clamp_lo, clamp_hi):
    """inv_h, s_bias: s = inv_h*x + s_bias (after clamping x to [clamp_lo, clamp_hi])."""
    nc = bacc.Bacc()
    x_ext = nc.declare_dram_parameter("x", [B_CORE, N_IN], f32, isOutput=False)
    w2_ext = nc.declare_dram_parameter("w2", [N_FEAT * N_IN, N_OUT], MM_DT, isOutput=False)
    wb_ext = nc.declare_dram_parameter("wb", [N_IN, N_OUT], MM_DT, isOutput=False)
    out_ext = nc.declare_dram_parameter("out", [B_CORE, N_OUT], f32, isOutput=True)

    n_windows = B_CORE // WINDOW
    n_bt = WINDOW // 128  # batch tiles per window

    with tile.TileContext(nc) as tc:
        with tc.tile_pool(name="wpool", bufs=1) as wpool, \
             tc.tile_pool(name="xpool", bufs=1) as xpool, \
             tc.tile_pool(name="xcpool", bufs=1) as xcpool, \
             tc.tile_pool(name="xtpool", bufs=1) as xtpool, \
             tc.tile_pool(name="fpool", bufs=1) as fpool, \
             tc.tile_pool(name="tmp", bufs=6) as tmp, \
             tc.tile_pool(name="opool", bufs=3) as opool, \
             tc.tile_pool(name="mpool", bufs=1) as mpool, \
             tc.tile_pool(name="psum_t", bufs=3, space="PSUM") as psum_t, \
             tc.tile_pool(name="psum_mm", bufs=2, space="PSUM") as psum_mm:

            ident = mpool.tile([128, 128], f32, tag="ident")
            make_identity(nc, ident[:])

            # per-partition bias constants for ACT Square: s_bias - k, k=1..4
            biases = mpool.tile([128, 4], f32, tag="biases")
            for k in range(1, 5):
                nc.vector.memset(biases[:, k - 1:k], float(s_bias - k))

            # resident weights
            w2_tiles = {}
            for fi in range(N_FEAT):
                for ic in range(N_IC):
                    t = wpool.tile([128, N_OUT], MM_DT, tag=f"w2_{fi}_{ic}")
                    r0 = fi * N_IN + ic * 128
                    nc.sync.dma_start(out=t[:], in_=w2_ext[r0:r0 + 128, :])
                    w2_tiles[(fi, ic)] = t
            wb_tiles = {}
            for ic in range(N_IC):
                t = wpool.tile([128, N_OUT], MM_DT, tag=f"wb_{ic}")
                nc.sync.dma_start(out=t[:], in_=wb_ext[ic * 128:(ic + 1) * 128, :])
                wb_tiles[ic] = t

            for w in range(n_windows):
                b0 = w * WINDOW
                # load x window in natural layout: partitions=batch%128
                xnat = xpool.tile([128, n_bt, N_IN], f32, tag="xnat")
                nc.sync.dma_start(
                    out=xnat[:],
                    in_=x_ext[b0:b0 + WINDOW, :].rearrange("(j p) i -> p j i", p=128),
                )

                # transpose to (i, b) layout; clamp-move to xc (f32) and cast-move
                # to xth (MM_DT) for the base matmul
                xc_tiles, xth_tiles = {}, {}
                for ic in range(N_IC):
                    xc = xcpool.tile([128, WINDOW], f32, tag=f"xc_{ic}")
                    xth = xtpool.tile([128, WINDOW], MM_DT, tag=f"xth_{ic}")
                    for half in range(WINDOW // 512):
                        pt = psum_t.tile([128, 512], f32, tag="tp")
                        for j in range(4):
                            bj = half * 4 + j
                            nc.tensor.transpose(
                                pt[:, j * 128:(j + 1) * 128],
                                xnat[:, bj, ic * 128:(ic + 1) * 128],
                                ident[:],
                            )
                        nc.vector.tensor_scalar(
                            out=xc[:, half * 512:(half + 1) * 512], in0=pt[:],
                            scalar1=clamp_lo, scalar2=clamp_hi,
                            op0=ALU.max, op1=ALU.min,
                        )
                        nc.scalar.activation(
                            xth[:, half * 512:(half + 1) * 512], pt[:], AF.Copy,
                        )
                    xc_tiles[ic] = xc
                    xth_tiles[ic] = xth

                # features: d_k = inv_h*xc + (s_bias - k); q_k = d_k^2;
                # L_k = max(d,0)*q -> feat(2k-2); R_k = min(d,0)*q -> feat(2k-1)
                feat = {}
                for ic in range(N_IC):
                    for k in range(1, 5):
                        d = tmp.tile([128, WINDOW], f32, tag="d")
                        nc.vector.tensor_scalar(
                            out=d[:], in0=xc_tiles[ic][:],
                            scalar1=inv_h, scalar2=float(s_bias - k),
                            op0=ALU.mult, op1=ALU.add,
                        )
                        q = tmp.tile([128, WINDOW], f32, tag="q")
                        nc.scalar.activation(
                            q[:], xc_tiles[ic][:], AF.Square,
                            bias=biases[:, k - 1:k], scale=inv_h,
                        )
                        fL = fpool.tile([128, WINDOW], MM_DT, tag=f"f_{2*k-2}_{ic}")
                        nc.vector.scalar_tensor_tensor(
                            out=fL[:], in0=d[:], scalar=0.0, in1=q[:],
                            op0=ALU.max, op1=ALU.mult,
                        )
                        fR = fpool.tile([128, WINDOW], MM_DT, tag=f"f_{2*k-1}_{ic}")
                        nc.vector.scalar_tensor_tensor(
                            out=fR[:], in0=d[:], scalar=0.0, in1=q[:],
                            op0=ALU.min, op1=ALU.mult,
                        )
                        feat[(2 * k - 2, ic)] = fL
                        feat[(2 * k - 1, ic)] = fR

                # matmuls per batch tile: base (4 chunks) + spline (32 chunks)
                for bt in range(n_bt):
                    bs = slice(bt * 128, (bt + 1) * 128)
                    acc_b = psum_mm.tile([128, N_OUT], f32, tag="accb")
                    for ic in range(N_IC):
                        nc.tensor.matmul(
                            acc_b[:], xth_tiles[ic][:, bs], wb_tiles[ic][:],
                            start=(ic == 0), stop=(ic == N_IC - 1),
                        )
                    acc_s = psum_mm.tile([128, N_OUT], f32, tag="accs")
                    n_chunks = N_FEAT * N_IC
                    ci = 0
                    for fi in range(N_FEAT):
                        for ic in range(N_IC):
                            nc.tensor.matmul(
                                acc_s[:], feat[(fi, ic)][:, bs], w2_tiles[(fi, ic)][:],
                                start=(ci == 0), stop=(ci == n_chunks - 1),
                            )
                            ci += 1
                    silu_t = opool.tile([128, N_OUT], f32, tag="silu")
                    nc.scalar.activation(silu_t[:], acc_b[:], AF.Silu)
                    out_t = opool.tile([128, N_OUT], f32, tag="out")
                    nc.vector.tensor_tensor(
                        out=out_t[:], in0=acc_s[:], in1=silu_t[:], op=ALU.add,
                    )
                    nc.sync.dma_start(
                        out=out_ext[b0 + bt * 128:b0 + (bt + 1) * 128, :],
                        in_=out_t[:],
                    )
    nc.compile()
    return nc


_NC_CACHE = {}


def _get_nc(inv_h, s_bias, clamp_lo, clamp_hi):
    key = (inv_h, s_bias, clamp_lo, clamp_hi, str(MM_DT))
    if key not in _NC_CACHE:
        _NC_CACHE[key] = _build_nc(inv_h, s_bias, clamp_lo, clamp_hi)
    return _NC_CACHE[key]


def _prepare_weights(coeff, base_weight, knots):
    M, t3, h = _solve_basis_map(np.asarray(knots, np.float64))
    # W2[(f, i), o] = sum_r M[f, r] * coeff[i, o, r]
    c64 = np.asarray(coeff, np.float64)                      # (i, o, r)
    w2 = np.einsum("fr,ior->fio", M, c64).reshape(N_FEAT * N_IN, N_OUT)
    np_dt = np.float16 if MM_DT is f16 else None
    if np_dt is None:
        import ml_dtypes
        np_dt = ml_dtypes.bfloat16
    w2 = w2.astype(np.float32).astype(np_dt)
    wb = np.asarray(base_weight, np.float32).astype(np_dt)
    inv_h = float(1.0 / h)
    s_bias = float(-t3 / h)
    clamp_lo = float(t3)
    clamp_hi = float(t3 + N_GRID * h)
    return w2, wb, inv_h, s_bias, clamp_lo, clamp_hi


def kernel(x, coeff, base_weight, knots):
    x = np.asarray(x, np.float32)
    assert x.shape == (BATCH, N_IN), x.shape
    w2, wb, inv_h, s_bias, clamp_lo, clamp_hi = _prepare_weights(
        coeff, base_weight, knots)
    nc = _get_nc(inv_h, s_bias, clamp_lo, clamp_hi)

    in_maps = []
    for c in range(N_CORES):
        in_maps.append({
            "x": x[c * B_CORE:(c + 1) * B_CORE],
            "w2": w2,
            "wb": wb,
        })
    results = run_bass_kernel_spmd(nc, in_maps, list(range(N_CORES))).results
    out = np.concatenate([results[c]["out"] for c in range(N_CORES)], axis=0)
    return out.astype(np.float32)
